# revision 15
# baseline (speedup 1.0000x reference)
"""GRAND graph-diffusion kernel for 8 Trainium2 NeuronCores.

Model (reference semantics):
    x0 = x_in @ enc_w + enc_b                     [N, H]
    kx = x0 @ wk_w + wk_b ; qx = x0 @ wq_w + wq_b
    A[u, v] = exp(kx[u] . qx[v] / H)  for (u, v) in edges, else 0
    A = A / rowsum(A)  (rows with sum 0 divide by 1)
    U = 0.75 I + 0.25 A ; x <- U x, 16 times ; out = x @ dec_w + dec_b

Sharding: rows of A/U are sharded across 8 cores (1024 rows each). Because a
duplicate edge (u, v) always writes the identical value exp(kx_u.qx_v/H), the
scatter in the reference is equivalent to a dense compute masked by the 0/1
adjacency pattern, which the host builds from the integer edge list (index
preprocessing only) and ships per-core as an fp16 mask.

Per core the kernel computes AT = exp(qxT^T kxT / H) * maskT in fp16 resident
in SBUF ([8192 cols, 1024 rows], transposed layout), derives row sums via a
ones-matmul, and then runs the diffusion steps entirely out of SBUF:
    yT[64,1024] = sum_kc x_chunk[128,64]^T @ UT_chunk[128,512]   (PSUM f32)
    yT = yT * (0.25/rowsum) + 0.75 * xT_local                    (f32 exact)
Each 512-row half of y is independently transposed to node-major fp16,
AllGathered (partition-major payload so the reload is 1KB-contiguous), and
reloaded while the other half's matmuls run. The next step's first 32 column
chunks depend only on the first half-gather, hiding collective latency.
The last step feeds the decoder directly (no collective).
"""

import math
import os
import sys

import numpy as np

sys.path.insert(0, "/opt/trn_rl_repo")

import concourse.bass as bass
import concourse.mybir as mybir
import concourse.tile as tile
from concourse import bacc
from concourse.bass import ts
from concourse.bass_utils import run_bass_kernel_spmd
from concourse.masks import make_identity

F32 = mybir.dt.float32
F16 = mybir.dt.float16

N = 8192        # nodes
D = 128         # input features
H = 64          # hidden
CLS = 40        # classes
NCORES = 8
NL = N // NCORES  # 1024 local rows
KC = N // 128     # 64 column chunks of 128
FD = 512          # matmul moving free dim
JH = 4            # chunks-per-rank in one gather half
W = JH * H        # 256: payload width per rank per half

_CACHE = {}

# column-chunk order: chunks living in the first gather half (j<4) come
# first, so the next step can begin before the second half-gather lands.
ORDER = [rk * 8 + j for j in range(8) for rk in range(8)]


def _build(steps: int):
    nc = bacc.Bacc(
        "TRN2", target_bir_lowering=False, debug=False, num_devices=NCORES
    )

    # ---- kernel I/O ----
    xinT = nc.dram_tensor("xinT", [D, N], F32, kind="ExternalInput")
    xinT_loc = nc.dram_tensor("xinT_loc", [D, NL], F32, kind="ExternalInput")
    enc_w = nc.dram_tensor("enc_w", [D, H], F32, kind="ExternalInput")
    enc_b_col = nc.dram_tensor("enc_b_col", [H, 1], F32, kind="ExternalInput")
    enc_b_nm = nc.dram_tensor("enc_b_nm", [128, H], F32, kind="ExternalInput")
    wk_w = nc.dram_tensor("wk_w", [H, H], F32, kind="ExternalInput")
    wk_b_col = nc.dram_tensor("wk_b_col", [H, 1], F32, kind="ExternalInput")
    wq_w = nc.dram_tensor("wq_w", [H, H], F32, kind="ExternalInput")
    wq_b_col = nc.dram_tensor("wq_b_col", [H, 1], F32, kind="ExternalInput")
    dec_w = nc.dram_tensor("dec_w", [H, CLS], F32, kind="ExternalInput")
    dec_b_nm = nc.dram_tensor("dec_b_nm", [128, CLS], F32, kind="ExternalInput")
    maskT = nc.dram_tensor("maskT", [N, NL], F16, kind="ExternalInput")
    out_loc = nc.dram_tensor("out_loc", [NL, CLS], F32, kind="ExternalOutput")

    ag_in = [
        nc.dram_tensor(f"ag_in{h}", [128, W], F16, kind="Internal")
        for h in range(2)
    ]
    ag_out = [
        nc.dram_tensor(
            f"ag_out{h}", [N // JH // 2, W], F16,
            kind="Internal", addr_space="Shared",
        )
        for h in range(2)
    ]

    with tile.TileContext(nc) as tc:
        _body(
            nc, tc, steps,
            xinT, xinT_loc, enc_w, enc_b_col, enc_b_nm,
            wk_w, wk_b_col, wq_w, wq_b_col, dec_w, dec_b_nm,
            maskT, out_loc, ag_in, ag_out,
        )

    nc.compile()
    return nc


def _body(
    nc, tc, steps,
    xinT, xinT_loc, enc_w, enc_b_col, enc_b_nm,
    wk_w, wk_b_col, wq_w, wq_b_col, dec_w, dec_b_nm,
    maskT, out_loc, ag_in, ag_out,
):
    mm = nc.tensor.matmul
    rg = [list(range(NCORES))]

    with (
        tc.tile_pool(name="persist", bufs=1) as pp,
        tc.tile_pool(name="work", bufs=2) as wp,
        tc.tile_pool(name="xin", bufs=4) as xinp,
        tc.tile_pool(name="xts", bufs=2) as xp,
        tc.tile_pool(name="ytp", bufs=1) as yp_pool,
        tc.tile_pool(name="ps_flex", bufs=4, space="PSUM") as ps_flex,
        tc.tile_pool(name="ps_sm", bufs=2, space="PSUM") as ps_sm,
        tc.tile_pool(name="ps_y", bufs=1, space="PSUM") as ps_y,
    ):
        # ---- persistent SBUF state ----
        UT = pp.tile([128, KC * NL], F16, tag="UT")       # 128 KiB/partition
        # node-major x, split by gather half: xh[0] holds chunks rk*8+j, j<4
        # (free layout (rk, j, h)), xh[1] the j>=4 chunks.
        xh = [
            pp.tile([128, 8 * W], F16, tag=f"xh{h}", name=f"xh{h}")
            for h in range(2)
        ]
        scale_bc = pp.tile([H, NL], F32, tag="scalebc")   # 0.25/rowsum bcast
        ident = pp.tile([128, 128], F32, tag="ident")
        make_identity(nc, ident[:])
        ones_p = pp.tile([128, 1], F16, tag="ones_p")
        nc.vector.memset(ones_p[:], 1.0)
        ones_h = pp.tile([1, H], F32, tag="ones_h")
        nc.vector.memset(ones_h[:], 1.0)

        enc_w_sb = pp.tile([D, H], F32, tag="encw")
        nc.sync.dma_start(enc_w_sb[:], enc_w.ap())
        enc_bc_sb = pp.tile([H, 1], F32, tag="encbc")
        nc.sync.dma_start(enc_bc_sb[:], enc_b_col.ap())
        enc_bn_sb = pp.tile([128, H], F32, tag="encbn")
        nc.sync.dma_start(enc_bn_sb[:], enc_b_nm.ap())
        wk_sb = pp.tile([H, H], F32, tag="wkw")
        nc.sync.dma_start(wk_sb[:], wk_w.ap())
        wkb_sb = pp.tile([H, 1], F32, tag="wkb")
        nc.sync.dma_start(wkb_sb[:], wk_b_col.ap())
        wq_sb = pp.tile([H, H], F32, tag="wqw")
        nc.sync.dma_start(wq_sb[:], wq_w.ap())
        wqb_sb = pp.tile([H, 1], F32, tag="wqb")
        nc.sync.dma_start(wqb_sb[:], wq_b_col.ap())
        dec_w_sb = pp.tile([H, CLS], F32, tag="decw")
        nc.sync.dma_start(dec_w_sb[:], dec_w.ap())
        dec_b_sb = pp.tile([128, CLS], F32, tag="decb")
        nc.sync.dma_start(dec_b_sb[:], dec_b_nm.ap())

        def x_lhsT(kc):
            rk, j = kc // 8, kc % 8
            h, jj = (0, j) if j < JH else (1, j - JH)
            off = (rk * JH + jj) * H
            return xh[h][:, off:off + H]

        # ================= setup phase =================
        with tc.tile_pool(name="setup", bufs=1) as sp:
            qxT = sp.tile([H, N], F16, tag="qxT")
            kxT_loc = sp.tile([H, NL], F16, tag="kxT")
            x0T_loc = sp.tile([H, NL], F32, tag="x0Tloc")
            inv = sp.tile([1, NL], F32, tag="inv")

            # fold the encoder into the q projection on device:
            #   qxT = (enc_w wq_w)^T xinT + (wq_w^T enc_b + wq_b)
            encT_ps = ps_flex.tile([H, D], F32, tag="flex")
            nc.tensor.transpose(encT_ps[:], enc_w_sb[:], ident[:])
            encT = sp.tile([H, D], F32, tag="encT")
            nc.vector.tensor_copy(encT[:], encT_ps[:])
            ewq_ps = ps_flex.tile([D, H], F32, tag="flex")
            mm(ewq_ps[:], encT[:], wq_sb[:], start=True, stop=True)
            ew_q = sp.tile([D, H], F32, tag="ewq")
            nc.vector.tensor_copy(ew_q[:], ewq_ps[:])
            qb2_ps = ps_flex.tile([H, 1], F32, tag="flex")
            mm(qb2_ps[:], wq_sb[:], enc_bc_sb[:], start=True, stop=True)
            qb2 = sp.tile([H, 1], F32, tag="qb2")
            nc.vector.tensor_tensor(
                qb2[:], qb2_ps[:], wqb_sb[:], op=mybir.AluOpType.add
            )

            # local feature-major x0 (fp32, exact path) and kxT (fp16)
            # first, so the merged encoder/A-build loop below can consume
            # kxT_loc immediately.
            for f in range(2):
                xc = xinp.tile([D, FD], F32, tag="xinc")
                nc.sync.dma_start(xc[:], xinT_loc.ap()[:, ts(f, FD)])
                ps = ps_flex.tile([H, FD], F32, tag="flex")
                mm(ps[:], enc_w_sb[:], xc[:], start=True, stop=True)
                nc.vector.tensor_scalar_add(
                    x0T_loc[:, ts(f, FD)], ps[:], enc_bc_sb[:]
                )
                psk = ps_flex.tile([H, FD], F32, tag="flex")
                mm(psk[:], wk_sb[:], x0T_loc[:, ts(f, FD)],
                   start=True, stop=True)
                nc.vector.tensor_scalar_add(
                    kxT_loc[:, ts(f, FD)], psk[:], wkb_sb[:]
                )

            # Merged encoder + A-build: stream xinT in [128, 512] chunks;
            # per chunk j build the node-major fp16 x0, the fp16 qxT row,
            # then immediately the four A-build column chunks that depend
            # on it: UT = exp(qxT^T kxT / H) * maskT. The encoder rides in
            # the shadow of the exp/mask pipeline.
            for j in range(N // FD):
                xc = xinp.tile([D, FD], F32, tag="xinc")
                nc.sync.dma_start(xc[:], xinT.ap()[:, ts(j, FD)])
                for s in range(FD // 128):
                    kc = j * (FD // 128) + s
                    ps = ps_sm.tile([128, H], F32, tag="small")
                    mm(ps[:], xc[:, ts(s, 128)], enc_w_sb[:],
                       start=True, stop=True)
                    nc.vector.tensor_tensor(
                        x_lhsT(kc), ps[:], enc_bn_sb[:],
                        op=mybir.AluOpType.add,
                    )
                psq = ps_flex.tile([H, FD], F32, tag="flex")
                mm(psq[:], ew_q[:], xc[:], start=True, stop=True)
                nc.vector.tensor_scalar_add(
                    qxT[:, ts(j, FD)], psq[:], qb2[:]
                )
                for kc in range(j * (FD // 128), (j + 1) * (FD // 128)):
                    mkc = wp.tile([128, NL], F16, tag="mask",
                                  name=f"mkc{kc}")
                    nc.sync.dma_start(
                        mkc[:], maskT.ap()[kc * 128:(kc + 1) * 128, :]
                    )
                    for f in range(2):
                        sc = ps_flex.tile([128, FD], F32, tag="flex")
                        mm(sc[:], qxT[:, ts(kc, 128)],
                           kxT_loc[:, ts(f, FD)], start=True, stop=True)
                        ut = UT[:, kc * NL + f * FD: kc * NL + (f + 1) * FD]
                        nc.scalar.activation(
                            ut, sc[:], mybir.ActivationFunctionType.Exp,
                            scale=1.0 / H,
                        )
                        nc.vector.tensor_tensor(
                            ut, ut, mkc[:, ts(f, FD)],
                            op=mybir.AluOpType.mult,
                        )

            # row sums as one dense PE block (no per-chunk stalls)
            rs_ps = [
                ps_sm.tile([1, FD], F32, tag="small", name=f"rs_ps{f}")
                for f in range(2)
            ]
            for kc in range(KC):
                for f in range(2):
                    mm(rs_ps[f][:], ones_p[:],
                       UT[:, kc * NL + f * FD: kc * NL + (f + 1) * FD],
                       start=(kc == 0), stop=(kc == KC - 1))

            # scale = 0.25 / max(rowsum, tiny), broadcast to 64 partitions
            for f in range(2):
                nc.vector.tensor_scalar_max(
                    inv[:, ts(f, FD)], rs_ps[f][:], 1e-30
                )
            nc.vector.reciprocal(inv[:], inv[:])
            nc.vector.tensor_scalar_mul(inv[:], inv[:], 0.25)
            for f in range(2):
                bp = ps_flex.tile([H, FD], F32, tag="flex")
                mm(bp[:], ones_h[:], inv[:, ts(f, FD)], start=True, stop=True)
                nc.vector.tensor_copy(scale_bc[:, ts(f, FD)], bp[:])

            xts_cur = xp.tile([H, NL], F32, tag="xts")
            nc.vector.tensor_scalar_mul(xts_cur[:], x0T_loc[:], 0.75)

        # ================= diffusion steps =================
        for step in range(steps):
            last = step == steps - 1
            yp = ps_y.tile([H, NL], F32, tag="ypsum")
            yT = yp_pool.tile([H, NL], F32, tag="yT")
            if not last:
                xts_nxt = xp.tile([H, NL], F32, tag="xts")

            def dve_tail(f):
                nc.vector.tensor_tensor(
                    yT[:, ts(f, FD)], yp[:, ts(f, FD)], scale_bc[:, ts(f, FD)],
                    op=mybir.AluOpType.mult,
                )
                nc.vector.tensor_tensor(
                    yT[:, ts(f, FD)], yT[:, ts(f, FD)], xts_cur[:, ts(f, FD)],
                    op=mybir.AluOpType.add,
                )
                if not last:
                    nc.vector.tensor_scalar_mul(
                        xts_nxt[:, ts(f, FD)], yT[:, ts(f, FD)], 0.75
                    )

            def tr_copy(f, yst, r):
                tp = ps_sm.tile([128, H], F32, tag="small", name=f"tp{f}_{r}")
                nc.tensor.transpose(
                    tp[:], yT[:, ts(JH * f + r, 128)], ident[0:H, 0:H]
                )
                nc.vector.tensor_copy(yst[:, ts(r, H)], tp[:])

            def gather(f, yst):
                nc.sync.dma_start(ag_in[f].ap(), yst[:])
                nc.gpsimd.collective_compute(
                    "AllGather", mybir.AluOpType.bypass,
                    replica_groups=rg,
                    ins=[ag_in[f].ap()], outs=[ag_out[f].ap()],
                )
                nc.sync.dma_start(
                    xh[f][:],
                    ag_out[f].ap().rearrange("(rk p) w -> p rk w", p=128),
                )

            # half 0 matmuls
            for i, kc in enumerate(ORDER):
                mm(yp[:, 0:FD], x_lhsT(kc), UT[:, kc * NL: kc * NL + FD],
                   start=(i == 0), stop=(i == KC - 1))
            dve_tail(0)
            # half 1 matmuls, half-0 transposes interleaved into the PE
            # stream once the half-0 DVE tail has had time to complete
            yst0 = wp.tile([128, W], F16, tag="yst", name=f"yst0_{step}")
            trs = 0
            for i, kc in enumerate(ORDER):
                mm(yp[:, FD:NL], x_lhsT(kc),
                   UT[:, kc * NL + FD: kc * NL + NL],
                   start=(i == 0), stop=(i == KC - 1))
                if not last and i >= 4 and (i - 4) % 2 == 0 and trs < JH:
                    tr_copy(0, yst0, trs)
                    trs += 1
            if not last:
                while trs < JH:
                    tr_copy(0, yst0, trs)
                    trs += 1
                gather(0, yst0)
            dve_tail(1)
            if not last:
                yst1 = wp.tile([128, W], F16, tag="yst", name=f"yst1_{step}")
                for r in range(JH):
                    tr_copy(1, yst1, r)
                gather(1, yst1)
                xts_cur = xts_nxt
            else:
                for r in range(8):
                    dp = ps_sm.tile([128, H], F32, tag="small")
                    mm(dp[:, 0:CLS], yT[:, ts(r, 128)], dec_w_sb[:],
                       start=True, stop=True)
                    dsb = wp.tile([128, CLS], F32, tag="dsb")
                    nc.vector.tensor_tensor(
                        dsb[:], dp[:, 0:CLS], dec_b_sb[:],
                        op=mybir.AluOpType.add,
                    )
                    nc.sync.dma_start(
                        out_loc.ap()[r * 128:(r + 1) * 128, :], dsb[:]
                    )


def _get(steps: int):
    if steps not in _CACHE:
        _CACHE[steps] = _build(steps)
    return _CACHE[steps]


def kernel(**inputs):
    x_in = np.asarray(inputs["x_in"], dtype=np.float32)
    enc_w = np.asarray(inputs["enc_w"], dtype=np.float32)
    enc_b = np.asarray(inputs["enc_b"], dtype=np.float32)
    wk_w = np.asarray(inputs["wk_w"], dtype=np.float32)
    wk_b = np.asarray(inputs["wk_b"], dtype=np.float32)
    wq_w = np.asarray(inputs["wq_w"], dtype=np.float32)
    wq_b = np.asarray(inputs["wq_b"], dtype=np.float32)
    dec_w = np.asarray(inputs["dec_w"], dtype=np.float32)
    dec_b = np.asarray(inputs["dec_b"], dtype=np.float32)
    edges = np.asarray(inputs["edges"], dtype=np.int32)
    T = int(np.asarray(inputs["T"]))
    steps = int(math.ceil(T / 0.25))

    nc = _get(steps)

    xinT = np.ascontiguousarray(x_in.T)  # [128, 8192]
    enc_b_col = np.ascontiguousarray(enc_b.reshape(H, 1))
    enc_b_nm = np.ascontiguousarray(np.tile(enc_b.reshape(1, H), (128, 1)))
    wk_b_col = np.ascontiguousarray(wk_b.reshape(H, 1))
    wq_b_col = np.ascontiguousarray(wq_b.reshape(H, 1))
    dec_b_nm = np.ascontiguousarray(np.tile(dec_b.reshape(1, CLS), (128, 1)))

    # per-core fp16 adjacency masks in transposed layout: maskT[c][v, u_local]
    u = edges[:, 0].astype(np.int64)
    v = edges[:, 1].astype(np.int64)
    core = u // NL
    r = u % NL
    masks = np.zeros((NCORES, N, NL), dtype=np.float16)
    masks[core, v, r] = np.float16(1.0)

    in_maps = []
    for c in range(NCORES):
        in_maps.append({
            "xinT": xinT,
            "xinT_loc": np.ascontiguousarray(xinT[:, c * NL:(c + 1) * NL]),
            "enc_w": enc_w,
            "enc_b_col": enc_b_col,
            "enc_b_nm": enc_b_nm,
            "wk_w": wk_w,
            "wk_b_col": wk_b_col,
            "wq_w": wq_w,
            "wq_b_col": wq_b_col,
            "dec_w": dec_w,
            "dec_b_nm": dec_b_nm,
            "maskT": np.ascontiguousarray(masks[c]),
        })

    res = run_bass_kernel_spmd(
        nc, in_maps, core_ids=list(range(NCORES)),
        trace=bool(int(os.environ.get("GRAND_TRACE", "0"))),
    )
    out = np.concatenate(
        [res.results[c]["out_loc"] for c in range(NCORES)], axis=0
    )
    kernel.last_results = res
    return out
